# revision 4
# baseline (speedup 1.0000x reference)
"""Causal self-attention (GQA + RoPE) TP-sharded over 8 trn2 NeuronCores.

Sharding: core c owns Q heads {2c, 2c+1} and KV head c//2 (GQA rep=4 means
both Q heads map to the same KV head). Each core computes its head-shard of
q/k/v projections + rotary + causal attention + a partial o_proj against its
256-column shard of Wo. The host sums the 8 partial outputs.

Layouts (per core):
  xT   [2048, 4096]  x transposed (contraction dim on partitions)
  qT/kT [128, 2048]  per head, head_dim on partitions (scores contraction)
  v_nat [128, 16, 128] natural [t, d] chunks via PE transpose (PV contraction)
  scores kept transposed [tk, tq]: softmax denom via ones-matmul on PE,
  no max subtraction (weights are 0.02-scale, scores are O(1), exp is safe).
All matmul operands are float32r (single-pass fp22 multiply, fp32 accumulate).
"""

import sys

try:
    import concourse.bass as bass  # noqa: F401
except ImportError:
    sys.path.insert(0, "/opt/trn_rl_repo")

import math
from contextlib import ExitStack

import numpy as np

import concourse.bass as bass
import concourse.mybir as mybir
import concourse.tile as tile
from concourse import bacc
from concourse.bass_utils import run_bass_kernel_spmd

F32 = mybir.dt.float32
F32R = mybir.dt.float32r

B, T, C = 2, 2048, 2048
BT = B * T
N_HEAD, N_KV_HEAD, HD = 16, 4, 128
ROTARY_BASE = 10000
N_CORES = 8
QSH = 2 * HD  # q output dims per core (2 heads)
SCALE = 1.0 / math.sqrt(HD)

TT = 512  # t-tile (moving-operand free size)
NT = T // TT  # t tiles per batch (4)
KC = C // 128  # contraction chunks for projections (16)


def _sin_cos_np():
    # mirror reference._sin_cos bit-for-bit (float32 throughout)
    pos = np.arange(T, dtype=np.float32)
    dim = np.arange(HD // 2, dtype=np.float32)
    freq = (np.float32(ROTARY_BASE) ** (dim / np.float32(HD / 2))).astype(np.float32)
    freq = np.concatenate([freq, freq])
    angles = pos[:, None] / freq[None, :]
    return np.sin(angles).astype(np.float32), np.cos(angles).astype(np.float32)


def build_kernel():
    nc = bacc.Bacc()
    xT = nc.dram_tensor("xT", [C, BT], F32R, kind="ExternalInput")
    wq = nc.dram_tensor("wq", [C, QSH], F32R, kind="ExternalInput")
    wk = nc.dram_tensor("wk", [C, HD], F32R, kind="ExternalInput")
    wv = nc.dram_tensor("wv", [C, HD], F32R, kind="ExternalInput")
    wo = nc.dram_tensor("wo", [QSH, C], F32R, kind="ExternalInput")
    cosd = nc.dram_tensor("cosd", [HD, T], F32, kind="ExternalInput")
    sind = nc.dram_tensor("sind", [HD, T], F32, kind="ExternalInput")  # sign-folded
    trid = nc.dram_tensor("trid", [128, 128], F32R, kind="ExternalInput")
    identd = nc.dram_tensor("identd", [128, 128], F32R, kind="ExternalInput")
    onesd = nc.dram_tensor("onesd", [128, 1], F32R, kind="ExternalInput")
    out = nc.dram_tensor("out", [BT, C], F32, kind="ExternalOutput")

    with ExitStack() as ctx:
        tc = ctx.enter_context(tile.TileContext(nc))
        consts = ctx.enter_context(tc.tile_pool(name="consts", bufs=1))
        xpool = ctx.enter_context(tc.tile_pool(name="xc", bufs=6))
        qkpool = ctx.enter_context(tc.tile_pool(name="qk", bufs=3))
        kpool = ctx.enter_context(tc.tile_pool(name="kT", bufs=2))
        vpool = ctx.enter_context(tc.tile_pool(name="vnat", bufs=2))
        vtpool = ctx.enter_context(tc.tile_pool(name="vt", bufs=2))
        tmppool = ctx.enter_context(tc.tile_pool(name="ropetmp", bufs=2))
        ppool = ctx.enter_context(tc.tile_pool(name="pT", bufs=3))
        ytpool = ctx.enter_context(tc.tile_pool(name="yT", bufs=3))
        rcpool = ctx.enter_context(tc.tile_pool(name="rcp", bufs=2))
        rbcpool = ctx.enter_context(tc.tile_pool(name="rbc", bufs=2))
        outpool = ctx.enter_context(tc.tile_pool(name="osb", bufs=2))

        proj_ps = ctx.enter_context(tc.tile_pool(name="proj_ps", bufs=4, space="PSUM"))
        s_ps = ctx.enter_context(tc.tile_pool(name="s_ps", bufs=2, space="PSUM"))
        y_ps = ctx.enter_context(tc.tile_pool(name="y_ps", bufs=1, space="PSUM"))
        rs_ps = ctx.enter_context(tc.tile_pool(name="rs_ps", bufs=1, space="PSUM"))

        # resident weights/constants
        wq_sb = consts.tile([128, KC, QSH], F32R)
        nc.sync.dma_start(out=wq_sb, in_=wq.ap().rearrange("(kc p) d -> p kc d", p=128))
        wk_sb = consts.tile([128, KC, HD], F32R)
        nc.sync.dma_start(out=wk_sb, in_=wk.ap().rearrange("(kc p) d -> p kc d", p=128))
        wv_sb = consts.tile([128, KC, HD], F32R)
        nc.sync.dma_start(out=wv_sb, in_=wv.ap().rearrange("(kc p) d -> p kc d", p=128))
        wo_sb = consts.tile([128, 2, C], F32R)
        nc.sync.dma_start(out=wo_sb, in_=wo.ap().rearrange("(h p) n -> p h n", p=128))
        cos_sb = consts.tile([HD, T], F32)
        nc.sync.dma_start(out=cos_sb, in_=cosd.ap())
        sin_sb = consts.tile([HD, T], F32)
        nc.sync.dma_start(out=sin_sb, in_=sind.ap())
        tri_sb = consts.tile([128, 128], F32R)
        nc.sync.dma_start(out=tri_sb, in_=trid.ap())
        id_sb = consts.tile([128, 128], F32R)
        nc.sync.dma_start(out=id_sb, in_=identd.ap())
        ones_sb = consts.tile([128, 1], F32R)
        nc.sync.dma_start(out=ones_sb, in_=onesd.ap())

        xT_ap = xT.ap()
        out_ap = out.ap()

        def rope_evac(dst, pj, tpos):
            """dst[:, :] = rope(pj) using cos/sin cols [tpos, tpos+TT)."""
            cs = cos_sb[:, tpos : tpos + TT]
            sn = sin_sb[:, tpos : tpos + TT]
            nc.vector.tensor_mul(dst, pj, cs)
            tmp = tmppool.tile([128, TT], F32)
            nc.vector.tensor_mul(tmp[0:64], pj[64:128], sn[0:64])
            nc.vector.tensor_mul(tmp[64:128], pj[0:64], sn[64:128])
            nc.vector.tensor_add(dst, dst, tmp)

        for b in range(B):
            # ---------------- projections for batch b ----------------
            qT = [qkpool.tile([128, T], F32R, tag="qT", name=f"qT_{b}_{h}") for h in range(2)]
            kT = kpool.tile([128, T], F32R)
            v_sb = vpool.tile([128, NT * 4, HD], F32R)

            for jt in range(NT):
                tcol = b * T + jt * TT
                tpos = jt * TT
                xc = [xpool.tile([128, TT], F32R, tag="xc", name=f"xc_{b}_{jt}_{kc}") for kc in range(KC)]
                for kc in range(KC):
                    nc.sync.dma_start(
                        out=xc[kc],
                        in_=xT_ap[128 * kc : 128 * kc + 128, tcol : tcol + TT],
                    )
                # all four projection outputs interleaved over one kc sweep
                pq = [proj_ps.tile([128, TT], F32, tag="proj", name=f"pq_{b}_{jt}_{h}") for h in range(2)]
                pk = proj_ps.tile([128, TT], F32, tag="proj")
                pv = proj_ps.tile([128, TT], F32, tag="proj")
                for kc in range(KC):
                    st, sp = (kc == 0), (kc == KC - 1)
                    for h in range(2):
                        nc.tensor.matmul(
                            pq[h],
                            wq_sb[:, kc, 128 * h : 128 * h + 128],
                            xc[kc],
                            start=st,
                            stop=sp,
                        )
                    nc.tensor.matmul(pk, wk_sb[:, kc, :], xc[kc], start=st, stop=sp)
                    nc.tensor.matmul(pv, wv_sb[:, kc, :], xc[kc], start=st, stop=sp)
                for h in range(2):
                    rope_evac(qT[h][:, tpos : tpos + TT], pq[h], tpos)
                rope_evac(kT[:, tpos : tpos + TT], pk, tpos)
                vt_sb = vtpool.tile([128, TT], F32R)
                nc.vector.tensor_copy(vt_sb, pv)
                for i in range(4):
                    vt_ps = s_ps.tile([128, 128], F32R, tag="s", name=f"vtp_{b}_{jt}_{i}")
                    nc.tensor.transpose(vt_ps, vt_sb[:, 128 * i : 128 * i + 128], id_sb)
                    nc.vector.tensor_copy(v_sb[:, 4 * jt + i, :], vt_ps)

            # ---------------- attention for batch b ----------------
            yT = [ytpool.tile([128, T], F32R, tag="yT", name=f"yT_{b}_{h}") for h in range(2)]
            for h in range(2):
                for j in range(NT):
                    chunks = [(c, 0) for c in range(4 * j)]
                    chunks = [(4 * j, 0)] + chunks + [(4 * j + m, 128 * m) for m in (1, 2, 3)]
                    nch = len(chunks)
                    yp = y_ps.tile([128, TT], F32, tag="y")
                    rp = rs_ps.tile([1, TT], F32, tag="rs")
                    for idx, (cch, off) in enumerate(chunks):
                        sT = s_ps.tile([128, TT], F32, tag="s")
                        nc.tensor.matmul(
                            sT[:, off:],
                            kT[:, 128 * cch : 128 * cch + 128],
                            qT[h][:, TT * j + off : TT * j + TT],
                            start=True,
                            stop=True,
                        )
                        pT = ppool.tile([128, TT], F32R, tag="p")
                        nc.scalar.activation(
                            out=pT[:, off:],
                            in_=sT[:, off:],
                            func=mybir.ActivationFunctionType.Exp,
                            scale=SCALE,
                        )
                        if cch >= 4 * j:  # diagonal block: causal triangle
                            nc.vector.tensor_mul(
                                pT[:, off : off + 128], pT[:, off : off + 128], tri_sb
                            )
                        nc.tensor.matmul(
                            yp[:, off:],
                            v_sb[:, cch, :],
                            pT[:, off:],
                            start=(idx == 0),
                            stop=(idx == nch - 1),
                        )
                        nc.tensor.matmul(
                            rp[:, off:],
                            ones_sb,
                            pT[:, off:],
                            start=(idx == 0),
                            stop=(idx == nch - 1),
                        )
                    rcp = rcpool.tile([1, TT], F32)
                    nc.vector.reciprocal(rcp, rp)
                    rbc = rbcpool.tile([128, TT], F32)
                    nc.gpsimd.partition_broadcast(rbc, rcp, channels=128)
                    nc.vector.tensor_mul(yT[h][:, TT * j : TT * j + TT], yp, rbc)

            # ---------------- partial o_proj for batch b ----------------
            for ts_ in range(T // 128):
                osb = outpool.tile([128, C], F32)
                for n in range(C // TT):
                    op = proj_ps.tile([128, TT], F32, tag="proj", name=f"op_{b}_{ts_}_{n}")
                    for h in range(2):
                        nc.tensor.matmul(
                            op,
                            yT[h][:, 128 * ts_ : 128 * ts_ + 128],
                            wo_sb[:, h, TT * n : TT * n + TT],
                            start=(h == 0),
                            stop=(h == 1),
                        )
                    if n % 2 == 0:
                        nc.scalar.copy(osb[:, TT * n : TT * n + TT], op)
                    else:
                        nc.vector.tensor_copy(osb[:, TT * n : TT * n + TT], op)
                row = b * T + 128 * ts_
                nc.sync.dma_start(out=out_ap[row : row + 128, :], in_=osb)

    nc.finalize()
    return nc


_NC_CACHE = None
TRACE = False
LAST_RESULTS = None


def _get_nc():
    global _NC_CACHE
    if _NC_CACHE is None:
        _NC_CACHE = build_kernel()
    return _NC_CACHE


def kernel(x, Wq, Wk, Wv, Wo):
    x = np.asarray(x, dtype=np.float32)
    Wq = np.asarray(Wq, dtype=np.float32)
    Wk = np.asarray(Wk, dtype=np.float32)
    Wv = np.asarray(Wv, dtype=np.float32)
    Wo = np.asarray(Wo, dtype=np.float32)

    xT = np.ascontiguousarray(x.reshape(BT, C).T)
    sin_, cos_ = _sin_cos_np()  # [T, 128]
    cosd = np.ascontiguousarray(cos_.T)
    sind = np.ascontiguousarray(sin_.T)
    sind[0:64] = -sind[0:64]  # sign-folded for rotate_half
    trid = np.triu(np.ones((128, 128), dtype=np.float32))
    identd = np.eye(128, dtype=np.float32)
    onesd = np.ones((128, 1), dtype=np.float32)

    core_ids = list(range(N_CORES))
    in_maps = []
    for c in core_ids:
        g = c // 2
        in_maps.append(
            {
                "xT": xT,
                "wq": np.ascontiguousarray(Wq[QSH * c : QSH * (c + 1)].T),
                "wk": np.ascontiguousarray(Wk[HD * g : HD * (g + 1)].T),
                "wv": np.ascontiguousarray(Wv[HD * g : HD * (g + 1)].T),
                "wo": np.ascontiguousarray(Wo[:, QSH * c : QSH * (c + 1)].T),
                "cosd": cosd,
                "sind": sind,
                "trid": trid,
                "identd": identd,
                "onesd": onesd,
            }
        )
    global LAST_RESULTS
    res = run_bass_kernel_spmd(_get_nc(), in_maps, core_ids, trace=TRACE)
    LAST_RESULTS = res
    total = res.results[0]["out"].astype(np.float32)
    for c in core_ids[1:]:
        total = total + res.results[c]["out"]
    return total.reshape(B, T, C)


# revision 5
# speedup vs baseline: 1.0077x; 1.0077x over previous
"""Causal self-attention (GQA + RoPE) TP-sharded over 8 trn2 NeuronCores.

Sharding: core c owns Q heads {2c, 2c+1} and KV head c//2 (GQA rep=4 means
both Q heads map to the same KV head). Each core computes its head-shard of
q/k/v projections + rotary + causal attention + a partial o_proj against its
256-column shard of Wo. The host sums the 8 partial outputs.

Layouts (per core):
  xT   [2048, 4096]  x transposed (contraction dim on partitions)
  qT/kT [128, 2048]  per head, head_dim on partitions (scores contraction)
  v_nat [128, 16, 128] natural [t, d] chunks via PE transpose (PV contraction)
  scores kept transposed [tk, tq]: softmax denom via ones-matmul on PE,
  no max subtraction (weights are 0.02-scale, scores are O(1), exp is safe).
All matmul operands are float32r (single-pass fp22 multiply, fp32 accumulate).
"""

import sys

try:
    import concourse.bass as bass  # noqa: F401
except ImportError:
    sys.path.insert(0, "/opt/trn_rl_repo")

import math
from contextlib import ExitStack

import numpy as np

import concourse.bass as bass
import concourse.mybir as mybir
import concourse.tile as tile
from concourse import bacc
from concourse.bass_utils import run_bass_kernel_spmd

F32 = mybir.dt.float32
F32R = mybir.dt.float32r

B, T, C = 2, 2048, 2048
BT = B * T
N_HEAD, N_KV_HEAD, HD = 16, 4, 128
ROTARY_BASE = 10000
N_CORES = 8
QSH = 2 * HD  # q output dims per core (2 heads)
SCALE = 1.0 / math.sqrt(HD)

TT = 512  # t-tile (moving-operand free size)
NT = T // TT  # t tiles per batch (4)
KC = C // 128  # contraction chunks for projections (16)


def _sin_cos_np():
    # mirror reference._sin_cos bit-for-bit (float32 throughout)
    pos = np.arange(T, dtype=np.float32)
    dim = np.arange(HD // 2, dtype=np.float32)
    freq = (np.float32(ROTARY_BASE) ** (dim / np.float32(HD / 2))).astype(np.float32)
    freq = np.concatenate([freq, freq])
    angles = pos[:, None] / freq[None, :]
    return np.sin(angles).astype(np.float32), np.cos(angles).astype(np.float32)


def build_kernel():
    nc = bacc.Bacc()
    xT = nc.dram_tensor("xT", [C, BT], F32R, kind="ExternalInput")
    wq = nc.dram_tensor("wq", [C, QSH], F32R, kind="ExternalInput")
    wk = nc.dram_tensor("wk", [C, HD], F32R, kind="ExternalInput")
    wv = nc.dram_tensor("wv", [C, HD], F32R, kind="ExternalInput")
    wo = nc.dram_tensor("wo", [QSH, C], F32R, kind="ExternalInput")
    cosd = nc.dram_tensor("cosd", [HD, T], F32, kind="ExternalInput")
    sind = nc.dram_tensor("sind", [HD, T], F32, kind="ExternalInput")  # sign-folded
    trid = nc.dram_tensor("trid", [128, 128], F32R, kind="ExternalInput")
    identd = nc.dram_tensor("identd", [128, 128], F32R, kind="ExternalInput")
    onesd = nc.dram_tensor("onesd", [128, 1], F32R, kind="ExternalInput")
    out = nc.dram_tensor("out", [BT, C], F32, kind="ExternalOutput")

    with ExitStack() as ctx:
        tc = ctx.enter_context(tile.TileContext(nc))
        consts = ctx.enter_context(tc.tile_pool(name="consts", bufs=1))
        xpool = ctx.enter_context(tc.tile_pool(name="xc", bufs=8))
        qkpool = ctx.enter_context(tc.tile_pool(name="qk", bufs=3))
        kpool = ctx.enter_context(tc.tile_pool(name="kT", bufs=2))
        vpool = ctx.enter_context(tc.tile_pool(name="vnat", bufs=2))
        vtpool = ctx.enter_context(tc.tile_pool(name="vt", bufs=2))
        tmppool = ctx.enter_context(tc.tile_pool(name="ropetmp", bufs=2))
        ppool = ctx.enter_context(tc.tile_pool(name="pT", bufs=3))
        ytpool = ctx.enter_context(tc.tile_pool(name="yT", bufs=3))
        rcpool = ctx.enter_context(tc.tile_pool(name="rcp", bufs=2))
        rbcpool = ctx.enter_context(tc.tile_pool(name="rbc", bufs=2))
        outpool = ctx.enter_context(tc.tile_pool(name="osb", bufs=2))

        proj_ps = ctx.enter_context(tc.tile_pool(name="proj_ps", bufs=4, space="PSUM"))
        s_ps = ctx.enter_context(tc.tile_pool(name="s_ps", bufs=3, space="PSUM"))
        y_ps = ctx.enter_context(tc.tile_pool(name="y_ps", bufs=1, space="PSUM"))

        # resident weights/constants (per-chunk tiles so the first matmuls
        # only wait on their own 128-row slice, not the whole weight DMA)
        wq_sb, wk_sb, wv_sb = [], [], []
        for kc in range(KC):
            r = slice(128 * kc, 128 * kc + 128)
            wq_sb.append(consts.tile([128, QSH], F32R, name=f"wq_{kc}"))
            nc.sync.dma_start(out=wq_sb[kc], in_=wq.ap()[r, :])
            wk_sb.append(consts.tile([128, HD], F32R, name=f"wk_{kc}"))
            nc.sync.dma_start(out=wk_sb[kc], in_=wk.ap()[r, :])
            wv_sb.append(consts.tile([128, HD], F32R, name=f"wv_{kc}"))
            nc.sync.dma_start(out=wv_sb[kc], in_=wv.ap()[r, :])
        wo_sb = consts.tile([128, 2, C], F32R)
        nc.sync.dma_start(out=wo_sb, in_=wo.ap().rearrange("(h p) n -> p h n", p=128))
        cos_sb = consts.tile([HD, T], F32)
        nc.sync.dma_start(out=cos_sb, in_=cosd.ap())
        sin_sb = consts.tile([HD, T], F32)
        nc.sync.dma_start(out=sin_sb, in_=sind.ap())
        tri_sb = consts.tile([128, 128], F32R)
        nc.sync.dma_start(out=tri_sb, in_=trid.ap())
        id_sb = consts.tile([128, 128], F32R)
        nc.sync.dma_start(out=id_sb, in_=identd.ap())
        ones_sb = consts.tile([128, 1], F32R)
        nc.sync.dma_start(out=ones_sb, in_=onesd.ap())

        xT_ap = xT.ap()
        out_ap = out.ap()

        def rope_evac(dst, pj, tpos):
            """dst[:, :] = rope(pj) using cos/sin cols [tpos, tpos+TT)."""
            cs = cos_sb[:, tpos : tpos + TT]
            sn = sin_sb[:, tpos : tpos + TT]
            nc.vector.tensor_mul(dst, pj, cs)
            tmp = tmppool.tile([128, TT], F32)
            nc.vector.tensor_mul(tmp[0:64], pj[64:128], sn[0:64])
            nc.vector.tensor_mul(tmp[64:128], pj[0:64], sn[64:128])
            nc.vector.tensor_add(dst, dst, tmp)

        for b in range(B):
            # ---------------- projections for batch b ----------------
            qT = [qkpool.tile([128, T], F32R, tag="qT", name=f"qT_{b}_{h}") for h in range(2)]
            kT = kpool.tile([128, T], F32R)
            v_sb = vpool.tile([128, NT * 4, HD], F32R)

            for jt in range(NT):
                tcol = b * T + jt * TT
                tpos = jt * TT
                xc = [xpool.tile([128, TT], F32R, tag="xc", name=f"xc_{b}_{jt}_{kc}") for kc in range(KC)]
                for kc in range(KC):
                    nc.sync.dma_start(
                        out=xc[kc],
                        in_=xT_ap[128 * kc : 128 * kc + 128, tcol : tcol + TT],
                    )
                # all four projection outputs interleaved over one kc sweep
                pq = [proj_ps.tile([128, TT], F32, tag="proj", name=f"pq_{b}_{jt}_{h}") for h in range(2)]
                pk = proj_ps.tile([128, TT], F32, tag="proj")
                pv = proj_ps.tile([128, TT], F32, tag="proj")
                for kc in range(KC):
                    st, sp = (kc == 0), (kc == KC - 1)
                    for h in range(2):
                        nc.tensor.matmul(
                            pq[h],
                            wq_sb[kc][:, 128 * h : 128 * h + 128],
                            xc[kc],
                            start=st,
                            stop=sp,
                        )
                    nc.tensor.matmul(pk, wk_sb[kc], xc[kc], start=st, stop=sp)
                    nc.tensor.matmul(pv, wv_sb[kc], xc[kc], start=st, stop=sp)
                for h in range(2):
                    rope_evac(qT[h][:, tpos : tpos + TT], pq[h], tpos)
                rope_evac(kT[:, tpos : tpos + TT], pk, tpos)
                vt_sb = vtpool.tile([128, TT], F32R)
                nc.vector.tensor_copy(vt_sb, pv)
                for i in range(4):
                    vt_ps = s_ps.tile([128, 128], F32R, tag="s", name=f"vtp_{b}_{jt}_{i}")
                    nc.tensor.transpose(vt_ps, vt_sb[:, 128 * i : 128 * i + 128], id_sb)
                    nc.vector.tensor_copy(v_sb[:, 4 * jt + i, :], vt_ps)

            # ---------------- attention for batch b ----------------
            yT = [ytpool.tile([128, T], F32R, tag="yT", name=f"yT_{b}_{h}") for h in range(2)]
            for h in range(2):
                for j in range(NT):
                    chunks = [(c, 0) for c in range(4 * j)]
                    chunks = [(4 * j, 0)] + chunks + [(4 * j + m, 128 * m) for m in (1, 2, 3)]
                    nch = len(chunks)
                    yp = y_ps.tile([128, TT], F32, tag="y")
                    rp = s_ps.tile([1, TT], F32, tag="s", name=f"rp_{b}_{h}_{j}")
                    for idx, (cch, off) in enumerate(chunks):
                        sT = s_ps.tile([128, TT], F32, tag="s")
                        nc.tensor.matmul(
                            sT[:, off:],
                            kT[:, 128 * cch : 128 * cch + 128],
                            qT[h][:, TT * j + off : TT * j + TT],
                            start=True,
                            stop=True,
                        )
                        pT = ppool.tile([128, TT], F32R, tag="p")
                        nc.scalar.activation(
                            out=pT[:, off:],
                            in_=sT[:, off:],
                            func=mybir.ActivationFunctionType.Exp,
                            scale=SCALE,
                        )
                        if cch >= 4 * j:  # diagonal block: causal triangle
                            nc.vector.tensor_mul(
                                pT[:, off : off + 128], pT[:, off : off + 128], tri_sb
                            )
                        nc.tensor.matmul(
                            yp[:, off:],
                            v_sb[:, cch, :],
                            pT[:, off:],
                            start=(idx == 0),
                            stop=(idx == nch - 1),
                        )
                        nc.tensor.matmul(
                            rp[:, off:],
                            ones_sb,
                            pT[:, off:],
                            start=(idx == 0),
                            stop=(idx == nch - 1),
                        )
                    ysl = yT[h][:, TT * j : TT * j + TT]
                    nc.scalar.copy(ysl, yp)  # releases the psum bank fast
                    rcp = rcpool.tile([1, TT], F32)
                    nc.vector.reciprocal(rcp, rp)
                    rbc = rbcpool.tile([128, TT], F32)
                    nc.gpsimd.partition_broadcast(rbc, rcp, channels=128)
                    nc.vector.tensor_mul(ysl, ysl, rbc)

            # ---------------- partial o_proj for batch b ----------------
            for ts_ in range(T // 128):
                osb = outpool.tile([128, C], F32)
                for n in range(C // TT):
                    op = proj_ps.tile([128, TT], F32, tag="proj", name=f"op_{b}_{ts_}_{n}")
                    for h in range(2):
                        nc.tensor.matmul(
                            op,
                            yT[h][:, 128 * ts_ : 128 * ts_ + 128],
                            wo_sb[:, h, TT * n : TT * n + TT],
                            start=(h == 0),
                            stop=(h == 1),
                        )
                    if n % 2 == 0:
                        nc.scalar.copy(osb[:, TT * n : TT * n + TT], op)
                    else:
                        nc.vector.tensor_copy(osb[:, TT * n : TT * n + TT], op)
                row = b * T + 128 * ts_
                nc.sync.dma_start(out=out_ap[row : row + 128, :], in_=osb)

    nc.finalize()
    return nc


_NC_CACHE = None
TRACE = False
LAST_RESULTS = None


def _get_nc():
    global _NC_CACHE
    if _NC_CACHE is None:
        _NC_CACHE = build_kernel()
    return _NC_CACHE


def kernel(x, Wq, Wk, Wv, Wo):
    x = np.asarray(x, dtype=np.float32)
    Wq = np.asarray(Wq, dtype=np.float32)
    Wk = np.asarray(Wk, dtype=np.float32)
    Wv = np.asarray(Wv, dtype=np.float32)
    Wo = np.asarray(Wo, dtype=np.float32)

    xT = np.ascontiguousarray(x.reshape(BT, C).T)
    sin_, cos_ = _sin_cos_np()  # [T, 128]
    cosd = np.ascontiguousarray(cos_.T)
    sind = np.ascontiguousarray(sin_.T)
    sind[0:64] = -sind[0:64]  # sign-folded for rotate_half
    trid = np.triu(np.ones((128, 128), dtype=np.float32))
    identd = np.eye(128, dtype=np.float32)
    onesd = np.ones((128, 1), dtype=np.float32)

    core_ids = list(range(N_CORES))
    in_maps = []
    for c in core_ids:
        g = c // 2
        in_maps.append(
            {
                "xT": xT,
                "wq": np.ascontiguousarray(Wq[QSH * c : QSH * (c + 1)].T),
                "wk": np.ascontiguousarray(Wk[HD * g : HD * (g + 1)].T),
                "wv": np.ascontiguousarray(Wv[HD * g : HD * (g + 1)].T),
                "wo": np.ascontiguousarray(Wo[:, QSH * c : QSH * (c + 1)].T),
                "cosd": cosd,
                "sind": sind,
                "trid": trid,
                "identd": identd,
                "onesd": onesd,
            }
        )
    global LAST_RESULTS
    res = run_bass_kernel_spmd(_get_nc(), in_maps, core_ids, trace=TRACE)
    LAST_RESULTS = res
    total = res.results[0]["out"].astype(np.float32)
    for c in core_ids[1:]:
        total = total + res.results[c]["out"]
    return total.reshape(B, T, C)
